# revision 2
# baseline (speedup 1.0000x reference)
"""Trainium2 Bass kernel for 8-head MultiHeadAttention (B=2, S=4096, E=512).

Sharding: 8 cores = 2 batches x 4 query-row chunks of 1024. Each core computes
all 8 heads for its (batch, q-range). Structure:
  - QK^T scores built transposed ([k partitions, q free]) as in the baseline.
  - softmax exp split across three engines: ACT (exact table exp) plus DVE and
    GPSIMD using a single-instruction Schraudolph bit-trick (int16 write
    bitcast to bf16), all masked multiplicatively afterward on DVE.
  - The attention-value matmul uses pt blocks as the stationary operand so the
    output lands as ctx[q partitions, d free] with a ones-column denominator:
    full 128-partition output halves the PE row count vs the [d, q] layout.
  - Wv is folded into Wo on the host (Wo' = Wo @ blockdiag(Wv)) so no V
    projection runs on device; normalization is a per-partition reciprocal
    plus a free-dim broadcast multiply straight into concat staging.
  - concat [q, e] is flipped to [e, q] via PE transposes against a host
    identity, then the output projection streams q rows per 128-q chunk.
"""
import sys
for _p in ('/root/.axon_site/_ro/trn_rl_repo', '/opt/trn_rl_repo'):
    if _p not in sys.path:
        sys.path.append(_p)

import numpy as np
import ml_dtypes

import concourse.bass as bass
import concourse.tile as tile
from concourse import bacc, mybir
from concourse import bass_utils

F32 = mybir.dt.float32
BF16 = mybir.dt.bfloat16
I16 = mybir.dt.int16
AF = mybir.ActivationFunctionType
ALU = mybir.AluOpType

N_CORES = 8
B, S, E, H, DH = 2, 4096, 512, 8, 64
QLEN = S // 4          # 1024 q rows per core
KC = S // 128          # 32 k chunks
QW = QLEN // 512       # 2 q windows of 512

# Schraudolph exp-as-bf16-bits: int16(x*EXPA + EXPB) bitcast bf16 ~ exp(x/8)
LOG2E = 1.4426950408889634
EXPA = 128.0 * LOG2E / 8.0
EXPB = 128.0 * (127.0 - 0.05735) + 0.5  # +0.5 compensates trunc-toward-zero

# exp engine split per 32-chunk window: 'a'=ACT exact, 'p'=Pool, 'v'=DVE
EXP_ENG = {}
_pool_kcs = {1, 4, 7, 9, 12, 15, 18, 20, 23, 26, 28, 30}
_dve_kcs = {13}
for _kc in range(KC):
    EXP_ENG[_kc] = 'p' if _kc in _pool_kcs else ('v' if _kc in _dve_kcs else 'a')

_CACHE = {}


def _build_module():
    nc = bacc.Bacc("TRN2", target_bir_lowering=False, debug=False,
                   enable_asserts=True, num_devices=N_CORES)

    xqT = nc.dram_tensor("xqT", [E, QLEN], BF16, kind="ExternalInput").ap()
    xkT = nc.dram_tensor("xkT", [E, S], BF16, kind="ExternalInput").ap()
    valp = nc.dram_tensor("valp", [S, H * 65], BF16, kind="ExternalInput").ap()
    maskT = nc.dram_tensor("maskT", [S, QLEN], BF16, kind="ExternalInput").ap()
    ident = nc.dram_tensor("ident", [128, 128], BF16, kind="ExternalInput").ap()
    wqT = nc.dram_tensor("wqT", [DH, DH], BF16, kind="ExternalInput").ap()
    wkT = nc.dram_tensor("wkT", [DH, DH], BF16, kind="ExternalInput").ap()
    woT = nc.dram_tensor("woT", [E, E], BF16, kind="ExternalInput").ap()
    bo_b = nc.dram_tensor("bo_b", [128, E], F32, kind="ExternalInput").ap()
    out = nc.dram_tensor("out", [QLEN, E], F32, kind="ExternalOutput").ap()

    with tile.TileContext(nc) as tc:
        _emit(tc, nc, xqT, xkT, valp, maskT, ident, wqT, wkT, woT, bo_b, out)

    nc.compile()
    return nc


def _emit(tc, nc, xqT, xkT, valp, maskT, ident, wqT, wkT, woT, bo_b, out):
    from contextlib import ExitStack
    ctx = ExitStack()
    const = ctx.enter_context(tc.tile_pool(name="const", bufs=1))
    kpool = ctx.enter_context(tc.tile_pool(name="kproj", bufs=1))
    qpool = ctx.enter_context(tc.tile_pool(name="qproj", bufs=2))
    xkst = ctx.enter_context(tc.tile_pool(name="xkst", bufs=2))
    ppool = ctx.enter_context(tc.tile_pool(name="p", bufs=4))
    rcpool = ctx.enter_context(tc.tile_pool(name="rc", bufs=2))
    ospool = ctx.enter_context(tc.tile_pool(name="osb", bufs=2))
    psp = ctx.enter_context(tc.tile_pool(name="psp", bufs=2, space="PSUM"))
    uacc = ctx.enter_context(tc.tile_pool(name="uacc", bufs=1, space="PSUM"))
    utmp = ctx.enter_context(tc.tile_pool(name="utmp", bufs=2, space="PSUM"))

    # ---- resident mask tiles (loaded once) ----
    mask_res = [const.tile([128, QLEN], BF16, tag=f"mk{c}", name=f"mk{c}")
                for c in range(KC)]

    def load_masks():
        for c in range(KC):
            nc.sync.dma_start(mask_res[c], maskT[c * 128:(c + 1) * 128, :])

    # ---- constants (wq/wk immediately; heavy/late consts after proj0 loads)
    wq_sb = const.tile([DH, DH], BF16, tag="wq")
    nc.gpsimd.dma_start(wq_sb, wqT)
    wk_sb = const.tile([DH, DH], BF16, tag="wk")
    nc.gpsimd.dma_start(wk_sb, wkT)
    ident_sb = const.tile([128, 128], BF16, tag="ident")
    nc.gpsimd.dma_start(ident_sb, ident)
    wo_sb = []
    for pc in range(4):
        wo_sb.append(const.tile([128, E], BF16, tag=f"wo{pc}", name=f"wo{pc}"))
    bo_sb = const.tile([128, E], F32, tag="bo")

    def load_late_consts():
        for pc in range(4):
            nc.sync.dma_start(wo_sb[pc], woT[pc * 128:(pc + 1) * 128, :])
        nc.sync.dma_start(bo_sb, bo_b)

    valp_t = []
    for c in range(KC):
        t = const.tile([128, H * 65], BF16, tag=f"vp{c}", name=f"vp{c}")
        nc.scalar.dma_start(t, valp[c * 128:(c + 1) * 128, :])
        valp_t.append(t)

    # concat staging [q-chunk 128, E] bf16, one per 128-q chunk of this core
    concat_q = [const.tile([128, E], BF16, tag=f"cq{c}", name=f"cq{c}")
                for c in range(8)]
    # transposed concat [e, q] for the out-projection lhsT: 4 pc-blocks x QLEN
    ct_sb = const.tile([128, 4 * QLEN], BF16, tag="ct")

    kproj_sb = [None] * 4
    qproj_sb = [None] * 4
    xs = {}

    def proj_load(pair):
        xk0 = xkst.tile([DH, S], BF16, tag="xk", name=f"xk0_{pair}")
        nc.sync.dma_start(xk0, xkT[(2 * pair) * DH:(2 * pair + 1) * DH, :])
        xq0 = xkst.tile([DH, QLEN], BF16, tag="xq", name=f"xq0_{pair}")
        nc.sync.dma_start(xq0, xqT[(2 * pair) * DH:(2 * pair + 1) * DH, :])
        xk1 = xkst.tile([DH, S], BF16, tag="xk", name=f"xk1_{pair}")
        nc.sync.dma_start(xk1, xkT[(2 * pair + 1) * DH:(2 * pair + 2) * DH, :])
        xq1 = xkst.tile([DH, QLEN], BF16, tag="xq", name=f"xq1_{pair}")
        nc.sync.dma_start(xq1, xqT[(2 * pair + 1) * DH:(2 * pair + 2) * DH, :])
        xs[pair] = (xk0, xk1, xq0, xq1)
        kproj_sb[pair] = kpool.tile([128, S], BF16, tag=f"kp{pair}",
                                    name=f"kp{pair}")
        qproj_sb[pair] = qpool.tile([128, QLEN], BF16, tag="qp",
                                    name=f"qp{pair}")

    def proj_chunks(pair, fast_start=False):
        """Closures: 8 kproj chunks + 2 qproj chunks. Copies run on ACT."""
        xk0, xk1, xq0, xq1 = xs[pair]
        kp_sb = kproj_sb[pair]
        qp_sb = qproj_sb[pair]

        def half(dst, srcs, w, h2, nm):
            def go():
                t = utmp.tile([128, 512], F32, tag="ut", name=nm)
                lo, hi = h2 * 64, (h2 + 1) * 64
                nc.tensor.matmul(t[lo:hi, :], lhsT=w, rhs=srcs[:, 0:512],
                                 start=True, stop=True)
                nc.scalar.copy(dst[lo:hi, 0:512], t[lo:hi, :])
            return go

        def kchunk(kc):
            def go():
                kp = utmp.tile([128, 512], F32, tag="ut", name=f"kpp{pair}_{kc}")
                nc.tensor.matmul(kp[0:64, :], lhsT=wk_sb,
                                 rhs=xk0[:, kc * 512:(kc + 1) * 512],
                                 start=True, stop=True)
                nc.tensor.matmul(kp[64:128, :], lhsT=wk_sb,
                                 rhs=xk1[:, kc * 512:(kc + 1) * 512],
                                 start=True, stop=True)
                nc.scalar.copy(kp_sb[:, kc * 512:(kc + 1) * 512], kp)
            return go

        def qchunk(qc):
            def go():
                qp = utmp.tile([128, 512], F32, tag="ut", name=f"qpp{pair}_{qc}")
                nc.tensor.matmul(qp[0:64, :], lhsT=wq_sb,
                                 rhs=xq0[:, qc * 512:(qc + 1) * 512],
                                 start=True, stop=True)
                nc.tensor.matmul(qp[64:128, :], lhsT=wq_sb,
                                 rhs=xq1[:, qc * 512:(qc + 1) * 512],
                                 start=True, stop=True)
                nc.scalar.copy(qp_sb[:, qc * 512:(qc + 1) * 512], qp)
            return go

        if fast_start:
            return ([half(kp_sb, xk0, wk_sb, 0, "fk0"),
                     half(qp_sb, xq0, wq_sb, 0, "fq0"),
                     half(kp_sb, xk1, wk_sb, 1, "fk1"),
                     half(qp_sb, xq1, wq_sb, 1, "fq1")]
                    + [kchunk(kc) for kc in range(1, 8)]
                    + [qchunk(qc) for qc in range(1, QW)])
        return ([kchunk(0), qchunk(0)] + [kchunk(kc) for kc in range(1, 8)]
                + [qchunk(qc) for qc in range(1, QW)])

    def attn(pair, qw, trickle=()):
        """One (head-pair, q-window): scores -> exp -> mask -> PV accumulate.

        Returns normalize closures (pure DVE) for trickling into the next
        phase. U layout: per head one PSUM bank [128 q, 4x(64 ctx + 1 denom)].
        """
        trickle = list(trickle)
        kp_sb = kproj_sb[pair]
        qp_sb = qproj_sb[pair]
        U = [uacc.tile([128, 260], F32, tag=f"u{h2}", name=f"U{pair}_{qw}_{h2}")
             for h2 in range(2)]
        for kc in range(KC):
            ps = psp.tile([128, 1024], F32, tag="ps", name=f"ps{pair}_{qw}_{kc}")
            nc.tensor.matmul(
                ps[:, 0:512],
                lhsT=kp_sb[0:64, kc * 128:(kc + 1) * 128],
                rhs=qp_sb[0:64, qw * 512:(qw + 1) * 512],
                start=True, stop=True)
            nc.tensor.matmul(
                ps[:, 512:1024],
                lhsT=kp_sb[64:128, kc * 128:(kc + 1) * 128],
                rhs=qp_sb[64:128, qw * 512:(qw + 1) * 512],
                start=True, stop=True)
            pt = ppool.tile([128, 1024], BF16, tag="pt", name=f"pt{pair}_{qw}_{kc}")
            eng = EXP_ENG[kc]
            if eng == 'a':
                nc.scalar.activation(pt, ps, AF.Exp, bias=0.0, scale=0.125)
            else:
                e = nc.vector if eng == 'v' else nc.gpsimd
                e.tensor_scalar(pt.bitcast(I16), ps, EXPA, EXPB,
                                ALU.mult, ALU.add)
            # mask multiply (DVE, bf16 2x), broadcast across the head pair
            ms = mask_res[kc][:, qw * 512:(qw + 1) * 512]
            mb = bass.AP(tensor=ms.tensor, offset=ms.offset,
                         ap=[ms.ap[0], [0, 2], [1, 512]])
            pv = pt.rearrange("p (h q) -> p h q", h=2)
            nc.vector.tensor_mul(pv, pv, mb)
            # PV: pt blocks stationary -> ctx [q, d] + denominator column.
            # One start=True per U bank marks the whole bank pending-zero;
            # each group's first write then zero-fills its own bytes.
            for h2 in range(2):
                h = 2 * pair + h2
                for qs in range(4):
                    nc.tensor.matmul(
                        U[h2][:, qs * 65:qs * 65 + 65],
                        lhsT=pt[:, h2 * 512 + qs * 128:h2 * 512 + (qs + 1) * 128],
                        rhs=valp_t[kc][:, h * 65:(h + 1) * 65],
                        start=(kc == 0 and qs == 0), stop=(kc == KC - 1),
                        skip_group_check=True)
            if trickle and kc >= 3:
                trickle.pop(0)()
                if trickle and kc >= 24:
                    trickle.pop(0)()
        for work in trickle:
            work()

        def norms():
            out_closures = []
            rc = [None, None]

            def recips(h2):
                def go():
                    rc[h2] = rcpool.tile([128, 4], F32, tag="rc",
                                         name=f"rc{pair}_{qw}_{h2}")
                    den = bass.AP(tensor=U[h2].tensor, offset=U[h2].offset + 64,
                                  ap=[U[h2].ap[0], [65, 4]])
                    nc.vector.reciprocal(rc[h2], den)
                return go

            def norm_one(h2, qs):
                def go():
                    h = 2 * pair + h2
                    cq = concat_q[qw * 4 + qs]
                    r = rc[h2]
                    rb = bass.AP(tensor=r.tensor, offset=r.offset + qs,
                                 ap=[r.ap[0], [0, 64]])
                    nc.vector.scalar_tensor_tensor(
                        cq[:, h * 64:(h + 1) * 64],
                        U[h2][:, qs * 65:qs * 65 + 64], 1.0, rb,
                        ALU.mult, ALU.mult)
                return go

            for h2 in range(2):
                out_closures.append(recips(h2))
                for qs in range(4):
                    out_closures.append(norm_one(h2, qs))
            return out_closures
        return norms()

    def finish_chunk(c):
        """Transpose concat chunk c into ct and run its out-projection."""
        def tr():
            ut = utmp.tile([128, 512], F32, tag="ut", name=f"tr{c}")
            ut_bf = ut.bitcast(BF16)
            for pc in range(4):
                nc.tensor.transpose(ut_bf[:, pc * 128:(pc + 1) * 128],
                                    concat_q[c][:, pc * 128:(pc + 1) * 128],
                                    ident_sb)
            ctv = bass.AP(tensor=ct_sb.tensor, offset=ct_sb.offset + c * 128,
                          ap=[ct_sb.ap[0], [QLEN, 4], [1, 128]])
            nc.vector.tensor_copy(ctv, ut_bf[:, 0:512])

        def op():
            o = utmp.tile([128, 512], F32, tag="ut", name=f"op{c}")
            for pc in range(4):
                nc.tensor.matmul(o,
                                 lhsT=ct_sb[:, pc * QLEN + c * 128:
                                            pc * QLEN + (c + 1) * 128],
                                 rhs=wo_sb[pc],
                                 start=(pc == 0), stop=(pc == 3))
            osb = ospool.tile([128, E], F32, tag="osb", name=f"osb{c}")
            nc.vector.scalar_tensor_tensor(osb, o, 1.0, bo_sb,
                                           ALU.mult, ALU.add)
            nc.sync.dma_start(out[c * 128:(c + 1) * 128, :], osb)
        return [tr, op]

    # emission schedule: proj0 upfront; later projections and every phase
    # tail trickle into the attention k-loops
    proj_load(0)
    for work in proj_chunks(0, fast_start=True):
        work()
    proj_load(1)
    load_masks()
    load_late_consts()
    t00 = attn(0, 0, trickle=proj_chunks(1))
    t01 = attn(0, 1, trickle=t00)
    proj_load(2)
    t10 = attn(1, 0, trickle=t01 + proj_chunks(2))
    t11 = attn(1, 1, trickle=t10)
    proj_load(3)
    t20 = attn(2, 0, trickle=t11 + proj_chunks(3))
    t21 = attn(2, 1, trickle=t20)
    t30 = attn(3, 0, trickle=t21)
    # t30 completes concat chunks 0..3 -> their transposes + outproj trickle
    fin03 = []
    for i, c in enumerate(range(4)):
        # keep order: norms for chunk c (inside t30) precede finish_chunk(c)
        fin03.extend(finish_chunk(c))
    t31 = attn(3, 1, trickle=t30 + fin03)
    for work in t31:
        work()
    for c in range(4, 8):
        for work in finish_chunk(c):
            work()

    ctx.close()


def _prep_inputs(key, query, value, mask, Wq, Wk, Wv, Wo, bo):
    bf16 = ml_dtypes.bfloat16
    key = np.asarray(key, np.float32)
    query = np.asarray(query, np.float32)
    value = np.asarray(value, np.float32)
    mask = np.asarray(mask)
    Wv = np.asarray(Wv, np.float32)
    Wo = np.asarray(Wo, np.float32)
    # fold the V projection into the output projection:
    # concat_h(ctxraw_h @ Wv^T) @ Wo^T == concat_raw @ (Wo @ blockdiag(Wv))^T
    Wof = np.empty_like(Wo)
    for h in range(H):
        Wof[:, h * DH:(h + 1) * DH] = Wo[:, h * DH:(h + 1) * DH] @ Wv
    common = {
        "wqT": np.ascontiguousarray(np.asarray(Wq, np.float32).T).astype(bf16),
        "wkT": np.ascontiguousarray(np.asarray(Wk, np.float32).T).astype(bf16),
        "woT": np.ascontiguousarray(Wof.T).astype(bf16),
        "bo_b": np.ascontiguousarray(
            np.broadcast_to(np.asarray(bo, np.float32), (128, E))),
        "ident": np.eye(128, dtype=np.float32).astype(bf16),
    }
    maskT = np.ascontiguousarray(
        (mask[0, 0] != 0).astype(np.float32).T.astype(bf16))  # [k, q]
    per_b = {}
    for b in range(B):
        vp = np.ones((S, H, 65), np.float32)
        vp[:, :, :64] = value[b].reshape(S, H, DH)
        per_b[b] = {
            "xkT": np.ascontiguousarray(key[b].T).astype(bf16),
            "valp": np.ascontiguousarray(vp.reshape(S, H * 65).astype(bf16)),
            "qT": query[b].T,
        }
    in_maps = []
    for c in range(N_CORES):
        b, qs = c // 4, (c % 4) * QLEN
        in_maps.append({
            "xqT": np.ascontiguousarray(
                per_b[b]["qT"][:, qs:qs + QLEN]).astype(bf16),
            "xkT": per_b[b]["xkT"],
            "valp": per_b[b]["valp"],
            "maskT": np.ascontiguousarray(maskT[:, qs:qs + QLEN]),
            **common,
        })
    return in_maps


def get_module():
    if "nc" not in _CACHE:
        _CACHE["nc"] = _build_module()
    return _CACHE["nc"]


def kernel(key, query, value, mask, Wq, Wk, Wv, Wo, bo, **_):
    nc = get_module()
    in_maps = _prep_inputs(key, query, value, mask, Wq, Wk, Wv, Wo, bo)
    res = bass_utils.run_bass_kernel_spmd(
        nc, in_maps, core_ids=list(range(N_CORES)))
    full = np.empty((B, S, E), np.float32)
    for c in range(N_CORES):
        b, qs = c // 4, (c % 4) * QLEN
        full[b, qs:qs + QLEN, :] = res.results[c]["out"]
    return full


# revision 7
# speedup vs baseline: 1.0929x; 1.0929x over previous
"""Trainium2 Bass kernel for 8-head MultiHeadAttention (B=2, S=4096, E=512).

Sharding: 8 cores = 2 batches x 4 query-row chunks of 1024. Each core computes
all 8 heads for its (batch, q-range). Structure:
  - QK^T scores built transposed ([k partitions, q free]) as in the baseline.
  - softmax exp split across three engines: ACT (exact table exp) plus DVE and
    GPSIMD using a single-instruction Schraudolph bit-trick (int16 write
    bitcast to bf16), all masked multiplicatively afterward on DVE.
  - The attention-value matmul uses pt blocks as the stationary operand so the
    output lands as ctx[q partitions, d free] with a ones-column denominator:
    full 128-partition output halves the PE row count vs the [d, q] layout.
  - Wv is folded into Wo on the host (Wo' = Wo @ blockdiag(Wv)) so no V
    projection runs on device; normalization is a per-partition reciprocal
    plus a free-dim broadcast multiply straight into concat staging.
  - concat [q, e] is flipped to [e, q] via PE transposes against a host
    identity, then the output projection streams q rows per 128-q chunk.
"""
import sys
for _p in ('/root/.axon_site/_ro/trn_rl_repo', '/opt/trn_rl_repo'):
    if _p not in sys.path:
        sys.path.append(_p)

import numpy as np
import ml_dtypes

import concourse.bass as bass
import concourse.tile as tile
from concourse import bacc, mybir
from concourse import bass_utils

F32 = mybir.dt.float32
BF16 = mybir.dt.bfloat16
I16 = mybir.dt.int16
AF = mybir.ActivationFunctionType
ALU = mybir.AluOpType

N_CORES = 8
B, S, E, H, DH = 2, 4096, 512, 8, 64
QLEN = S // 4          # 1024 q rows per core
KC = S // 128          # 32 k chunks
QW = QLEN // 512       # 2 q windows of 512

# Schraudolph exp-as-bf16-bits: int16(x*EXPA + EXPB) bitcast bf16 ~ exp(x/8)
LOG2E = 1.4426950408889634
EXPA = 128.0 * LOG2E / 8.0
EXPB = 128.0 * (127.0 - 0.05735) + 0.5  # +0.5 compensates trunc-toward-zero

# exp engine split per 32-chunk window: 'a'=ACT exact, 'p'=Pool, 'v'=DVE
EXP_ENG = {}
_pool_kcs = {1, 4, 7, 9, 12, 15, 18, 20, 23, 26, 28, 30}
_dve_kcs = {13}
for _kc in range(KC):
    EXP_ENG[_kc] = 'p' if _kc in _pool_kcs else ('v' if _kc in _dve_kcs else 'a')

_CACHE = {}


def _build_module():
    nc = bacc.Bacc("TRN2", target_bir_lowering=False, debug=False,
                   enable_asserts=True, num_devices=N_CORES)

    xqT = nc.dram_tensor("xqT", [E, QLEN], BF16, kind="ExternalInput").ap()
    xkT = nc.dram_tensor("xkT", [E, S], BF16, kind="ExternalInput").ap()
    valp = nc.dram_tensor("valp", [S, H * 65], BF16, kind="ExternalInput").ap()
    maskT = nc.dram_tensor("maskT", [S, QLEN], BF16, kind="ExternalInput").ap()
    ident = nc.dram_tensor("ident", [128, 128], BF16, kind="ExternalInput").ap()
    wqT = nc.dram_tensor("wqT", [DH, DH], BF16, kind="ExternalInput").ap()
    wkT = nc.dram_tensor("wkT", [DH, DH], BF16, kind="ExternalInput").ap()
    woT = nc.dram_tensor("woT", [E, E], BF16, kind="ExternalInput").ap()
    bo_b = nc.dram_tensor("bo_b", [128, E], F32, kind="ExternalInput").ap()
    out = nc.dram_tensor("out", [QLEN, E], F32, kind="ExternalOutput").ap()

    with tile.TileContext(nc) as tc:
        _emit(tc, nc, xqT, xkT, valp, maskT, ident, wqT, wkT, woT, bo_b, out)

    nc.compile()
    return nc


def _emit(tc, nc, xqT, xkT, valp, maskT, ident, wqT, wkT, woT, bo_b, out):
    from contextlib import ExitStack
    ctx = ExitStack()
    const = ctx.enter_context(tc.tile_pool(name="const", bufs=1))
    kpool = ctx.enter_context(tc.tile_pool(name="kproj", bufs=1))
    qpool = ctx.enter_context(tc.tile_pool(name="qproj", bufs=2))
    xkst = ctx.enter_context(tc.tile_pool(name="xkst", bufs=2))
    ppool = ctx.enter_context(tc.tile_pool(name="p", bufs=8))
    rcpool = ctx.enter_context(tc.tile_pool(name="rc", bufs=2))
    ospool = ctx.enter_context(tc.tile_pool(name="osb", bufs=2))
    psp = ctx.enter_context(tc.tile_pool(name="psp", bufs=3, space="PSUM"))
    uacc = ctx.enter_context(tc.tile_pool(name="uacc", bufs=1, space="PSUM"))

    def pstile(nm):
        """Scratch PSUM [128, 512] carved from the shared ps rotation."""
        t = psp.tile([128, 1024], F32, tag="ps", name=nm)
        return t[:, 0:512]

    # ---- resident mask tiles (loaded once) ----
    mask_res = [const.tile([128, QLEN], BF16, tag=f"mk{c}", name=f"mk{c}")
                for c in range(KC)]

    def load_masks():
        for c in range(KC):
            nc.sync.dma_start(mask_res[c], maskT[c * 128:(c + 1) * 128, :])

    # ---- constants (wq/wk immediately; heavy/late consts after proj0 loads)
    wq_sb = const.tile([DH, DH], BF16, tag="wq")
    nc.gpsimd.dma_start(wq_sb, wqT)
    wk_sb = const.tile([DH, DH], BF16, tag="wk")
    nc.gpsimd.dma_start(wk_sb, wkT)
    ident_sb = const.tile([128, 128], BF16, tag="ident")
    nc.gpsimd.dma_start(ident_sb, ident)
    wo_sb = []
    for pc in range(4):
        wo_sb.append(const.tile([128, E], BF16, tag=f"wo{pc}", name=f"wo{pc}"))
    bo_sb = const.tile([128, E], F32, tag="bo")

    def load_late_consts():
        for pc in range(4):
            nc.sync.dma_start(wo_sb[pc], woT[pc * 128:(pc + 1) * 128, :])
        nc.sync.dma_start(bo_sb, bo_b)

    valp_t = []
    for c in range(KC):
        t = const.tile([128, H * 65], BF16, tag=f"vp{c}", name=f"vp{c}")
        nc.scalar.dma_start(t, valp[c * 128:(c + 1) * 128, :])
        valp_t.append(t)

    # concat staging [q-chunk 128, E] bf16, one per 128-q chunk of this core
    concat_q = [const.tile([128, E], BF16, tag=f"cq{c}", name=f"cq{c}")
                for c in range(8)]
    # transposed concat [e, q] for the out-projection lhsT: 4 pc-blocks x QLEN
    ct_sb = const.tile([128, 4 * QLEN], BF16, tag="ct")

    kproj_sb = [None] * 4
    qproj_sb = [None] * 4
    xs = {}

    def proj_load(pair):
        xk0 = xkst.tile([DH, S], BF16, tag="xk", name=f"xk0_{pair}")
        nc.sync.dma_start(xk0, xkT[(2 * pair) * DH:(2 * pair + 1) * DH, :])
        xq0 = xkst.tile([DH, QLEN], BF16, tag="xq", name=f"xq0_{pair}")
        nc.sync.dma_start(xq0, xqT[(2 * pair) * DH:(2 * pair + 1) * DH, :])
        xk1 = xkst.tile([DH, S], BF16, tag="xk", name=f"xk1_{pair}")
        nc.sync.dma_start(xk1, xkT[(2 * pair + 1) * DH:(2 * pair + 2) * DH, :])
        xq1 = xkst.tile([DH, QLEN], BF16, tag="xq", name=f"xq1_{pair}")
        nc.sync.dma_start(xq1, xqT[(2 * pair + 1) * DH:(2 * pair + 2) * DH, :])
        xs[pair] = (xk0, xk1, xq0, xq1)
        kproj_sb[pair] = kpool.tile([128, S], BF16, tag=f"kp{pair}",
                                    name=f"kp{pair}")
        qproj_sb[pair] = qpool.tile([128, QLEN], BF16, tag="qp",
                                    name=f"qp{pair}")

    def proj_chunks(pair, fast_start=False):
        """Closures: 8 kproj chunks + 2 qproj chunks. Copies run on ACT."""
        xk0, xk1, xq0, xq1 = xs[pair]
        kp_sb = kproj_sb[pair]
        qp_sb = qproj_sb[pair]

        def half(dst, srcs, w, h2, nm):
            def go():
                t = pstile(nm)
                lo, hi = h2 * 64, (h2 + 1) * 64
                nc.tensor.matmul(t[lo:hi, :], lhsT=w, rhs=srcs[:, 0:512],
                                 start=True, stop=True)
                nc.scalar.copy(dst[lo:hi, 0:512], t[lo:hi, :])
            return go

        def kchunk(kc):
            def go():
                kp = pstile(f"kpp{pair}_{kc}")
                nc.tensor.matmul(kp[0:64, :], lhsT=wk_sb,
                                 rhs=xk0[:, kc * 512:(kc + 1) * 512],
                                 start=True, stop=True)
                nc.tensor.matmul(kp[64:128, :], lhsT=wk_sb,
                                 rhs=xk1[:, kc * 512:(kc + 1) * 512],
                                 start=True, stop=True)
                nc.scalar.copy(kp_sb[:, kc * 512:(kc + 1) * 512], kp)
            return go

        def qchunk(qc):
            def go():
                qp = pstile(f"qpp{pair}_{qc}")
                nc.tensor.matmul(qp[0:64, :], lhsT=wq_sb,
                                 rhs=xq0[:, qc * 512:(qc + 1) * 512],
                                 start=True, stop=True)
                nc.tensor.matmul(qp[64:128, :], lhsT=wq_sb,
                                 rhs=xq1[:, qc * 512:(qc + 1) * 512],
                                 start=True, stop=True)
                nc.scalar.copy(qp_sb[:, qc * 512:(qc + 1) * 512], qp)
            return go

        if fast_start:
            return ([half(kp_sb, xk0, wk_sb, 0, "fk0"),
                     half(qp_sb, xq0, wq_sb, 0, "fq0"),
                     half(kp_sb, xk1, wk_sb, 1, "fk1"),
                     half(qp_sb, xq1, wq_sb, 1, "fq1")]
                    + [kchunk(kc) for kc in range(1, 8)]
                    + [qchunk(qc) for qc in range(1, QW)])
        return ([kchunk(0), qchunk(0)] + [kchunk(kc) for kc in range(1, 8)]
                + [qchunk(qc) for qc in range(1, QW)])

    def attn(pair, qw, trickle=()):
        """One (head-pair, q-window): scores -> exp -> mask -> PV accumulate.

        Returns normalize closures (pure DVE) for trickling into the next
        phase. U layout: per head one PSUM bank [128 q, 4x(64 ctx + 1 denom)].
        """
        trickle = list(trickle)
        kp_sb = kproj_sb[pair]
        qp_sb = qproj_sb[pair]
        U = [uacc.tile([128, 260], F32, tag=f"u{h2}", name=f"U{pair}_{qw}_{h2}")
             for h2 in range(2)]
        SKEW = 6
        pts = {}
        for kc in range(KC + SKEW):
            if kc < KC:
                ps = psp.tile([128, 1024], F32, tag="ps",
                              name=f"ps{pair}_{qw}_{kc}")
                nc.tensor.matmul(
                    ps[:, 0:512],
                    lhsT=kp_sb[0:64, kc * 128:(kc + 1) * 128],
                    rhs=qp_sb[0:64, qw * 512:(qw + 1) * 512],
                    start=True, stop=True)
                nc.tensor.matmul(
                    ps[:, 512:1024],
                    lhsT=kp_sb[64:128, kc * 128:(kc + 1) * 128],
                    rhs=qp_sb[64:128, qw * 512:(qw + 1) * 512],
                    start=True, stop=True)
                pt = ppool.tile([128, 1024], BF16, tag="pt",
                                name=f"pt{pair}_{qw}_{kc}")
                pts[kc] = pt
                eng = EXP_ENG[kc]
                if eng == 'a':
                    nc.scalar.activation(pt, ps, AF.Exp, bias=0.0, scale=0.125)
                else:
                    e = nc.vector if eng == 'v' else nc.gpsimd
                    e.tensor_scalar(pt.bitcast(I16), ps, EXPA, EXPB,
                                    ALU.mult, ALU.add)
                # mask multiply (DVE, bf16 2x), broadcast across the head pair
                ms = mask_res[kc][:, qw * 512:(qw + 1) * 512]
                mb = bass.AP(tensor=ms.tensor, offset=ms.offset,
                             ap=[ms.ap[0], [0, 2], [1, 512]])
                pv = pt.rearrange("p (h q) -> p h q", h=2)
                nc.vector.tensor_mul(pv, pv, mb)
            if kc >= SKEW:
                # PV, skewed so PE never waits on this chunk's exp+mask.
                # One start=True per U bank marks the whole bank pending-zero;
                # each group's first write then zero-fills its own bytes.
                kv = kc - SKEW
                pt = pts.pop(kv)
                for h2 in range(2):
                    h = 2 * pair + h2
                    for qs in range(4):
                        nc.tensor.matmul(
                            U[h2][:, qs * 65:qs * 65 + 65],
                            lhsT=pt[:, h2 * 512 + qs * 128:
                                    h2 * 512 + (qs + 1) * 128],
                            rhs=valp_t[kv][:, h * 65:(h + 1) * 65],
                            start=(kv == 0 and qs == 0), stop=(kv == KC - 1),
                            skip_group_check=True)
            if trickle and kc >= 2:
                trickle.pop(0)()
                if trickle and kc >= 26:
                    trickle.pop(0)()
        for work in trickle:
            work()

        def norms():
            out_closures = []
            rc = [None, None]

            def recips(h2):
                def go():
                    rc[h2] = rcpool.tile([128, 4], F32, tag="rc",
                                         name=f"rc{pair}_{qw}_{h2}")
                    den = bass.AP(tensor=U[h2].tensor, offset=U[h2].offset + 64,
                                  ap=[U[h2].ap[0], [65, 4]])
                    nc.vector.reciprocal(rc[h2], den)
                return go

            def norm_one(h2, qs):
                def go():
                    h = 2 * pair + h2
                    cq = concat_q[qw * 4 + qs]
                    r = rc[h2]
                    rb = bass.AP(tensor=r.tensor, offset=r.offset + qs,
                                 ap=[r.ap[0], [0, 64]])
                    nc.vector.scalar_tensor_tensor(
                        cq[:, h * 64:(h + 1) * 64],
                        U[h2][:, qs * 65:qs * 65 + 64], 1.0, rb,
                        ALU.mult, ALU.mult)
                return go

            for h2 in range(2):
                out_closures.append(recips(h2))
                for qs in range(4):
                    out_closures.append(norm_one(h2, qs))
            return out_closures
        return norms()

    def finish_chunk(c):
        """Transpose concat chunk c into ct and run its out-projection."""
        def tr():
            ut = pstile(f"tr{c}")
            ut_bf = ut.bitcast(BF16)
            for pc in range(4):
                nc.tensor.transpose(ut_bf[:, pc * 128:(pc + 1) * 128],
                                    concat_q[c][:, pc * 128:(pc + 1) * 128],
                                    ident_sb)
            ctv = bass.AP(tensor=ct_sb.tensor, offset=ct_sb.offset + c * 128,
                          ap=[ct_sb.ap[0], [QLEN, 4], [1, 128]])
            nc.vector.tensor_copy(ctv, ut_bf[:, 0:512])

        def op():
            o = pstile(f"op{c}")
            for pc in range(4):
                nc.tensor.matmul(o,
                                 lhsT=ct_sb[:, pc * QLEN + c * 128:
                                            pc * QLEN + (c + 1) * 128],
                                 rhs=wo_sb[pc],
                                 start=(pc == 0), stop=(pc == 3))
            osb = ospool.tile([128, E], F32, tag="osb", name=f"osb{c}")
            nc.vector.scalar_tensor_tensor(osb, o, 1.0, bo_sb,
                                           ALU.mult, ALU.add)
            nc.sync.dma_start(out[c * 128:(c + 1) * 128, :], osb)
        return [tr, op]

    # emission schedule: proj0 upfront; later projections and every phase
    # tail trickle into the attention k-loops
    proj_load(0)
    for work in proj_chunks(0, fast_start=True):
        work()
    proj_load(1)
    load_masks()
    load_late_consts()
    t00 = attn(0, 0, trickle=proj_chunks(1))
    t01 = attn(0, 1, trickle=t00)
    proj_load(2)
    t10 = attn(1, 0, trickle=t01 + proj_chunks(2))
    t11 = attn(1, 1, trickle=t10)
    proj_load(3)
    t20 = attn(2, 0, trickle=t11 + proj_chunks(3))
    t21 = attn(2, 1, trickle=t20)
    t30 = attn(3, 0, trickle=t21)
    # t30 completes concat chunks 0..3 -> their transposes + outproj trickle
    fin03 = []
    for i, c in enumerate(range(4)):
        # keep order: norms for chunk c (inside t30) precede finish_chunk(c)
        fin03.extend(finish_chunk(c))
    t31 = attn(3, 1, trickle=t30 + fin03)
    for work in t31:
        work()
    for c in range(4, 8):
        for work in finish_chunk(c):
            work()

    ctx.close()


def _prep_inputs(key, query, value, mask, Wq, Wk, Wv, Wo, bo):
    bf16 = ml_dtypes.bfloat16
    key = np.asarray(key, np.float32)
    query = np.asarray(query, np.float32)
    value = np.asarray(value, np.float32)
    mask = np.asarray(mask)
    Wv = np.asarray(Wv, np.float32)
    Wo = np.asarray(Wo, np.float32)
    # fold the V projection into the output projection:
    # concat_h(ctxraw_h @ Wv^T) @ Wo^T == concat_raw @ (Wo @ blockdiag(Wv))^T
    Wof = np.empty_like(Wo)
    for h in range(H):
        Wof[:, h * DH:(h + 1) * DH] = Wo[:, h * DH:(h + 1) * DH] @ Wv
    common = {
        "wqT": np.ascontiguousarray(np.asarray(Wq, np.float32).T).astype(bf16),
        "wkT": np.ascontiguousarray(np.asarray(Wk, np.float32).T).astype(bf16),
        "woT": np.ascontiguousarray(Wof.T).astype(bf16),
        "bo_b": np.ascontiguousarray(
            np.broadcast_to(np.asarray(bo, np.float32), (128, E))),
        "ident": np.eye(128, dtype=np.float32).astype(bf16),
    }
    maskT = np.ascontiguousarray(
        (mask[0, 0] != 0).astype(np.float32).T.astype(bf16))  # [k, q]
    per_b = {}
    for b in range(B):
        vp = np.ones((S, H, 65), np.float32)
        vp[:, :, :64] = value[b].reshape(S, H, DH)
        per_b[b] = {
            "xkT": np.ascontiguousarray(key[b].T).astype(bf16),
            "valp": np.ascontiguousarray(vp.reshape(S, H * 65).astype(bf16)),
            "qT": query[b].T,
        }
    in_maps = []
    for c in range(N_CORES):
        b, qs = c // 4, (c % 4) * QLEN
        in_maps.append({
            "xqT": np.ascontiguousarray(
                per_b[b]["qT"][:, qs:qs + QLEN]).astype(bf16),
            "xkT": per_b[b]["xkT"],
            "valp": per_b[b]["valp"],
            "maskT": np.ascontiguousarray(maskT[:, qs:qs + QLEN]),
            **common,
        })
    return in_maps


def get_module():
    if "nc" not in _CACHE:
        _CACHE["nc"] = _build_module()
    return _CACHE["nc"]


def kernel(key, query, value, mask, Wq, Wk, Wv, Wo, bo, **_):
    nc = get_module()
    in_maps = _prep_inputs(key, query, value, mask, Wq, Wk, Wv, Wo, bo)
    res = bass_utils.run_bass_kernel_spmd(
        nc, in_maps, core_ids=list(range(N_CORES)))
    full = np.empty((B, S, E), np.float32)
    for c in range(N_CORES):
        b, qs = c // 4, (c % 4) * QLEN
        full[b, qs:qs + QLEN, :] = res.results[c]["out"]
    return full


# revision 18
# speedup vs baseline: 1.1794x; 1.0791x over previous
"""Trainium2 Bass kernel for 8-head MultiHeadAttention (B=2, S=4096, E=512).

Sharding: 8 cores = 2 batches x 4 query-row chunks of 1024. Each core computes
all 8 heads for its (batch, q-range). Structure:
  - QK^T scores built transposed ([k partitions, q free]) as in the baseline.
  - softmax exp split across three engines: ACT (exact table exp) plus DVE and
    GPSIMD using a single-instruction Schraudolph bit-trick (int16 write
    bitcast to bf16), all masked multiplicatively afterward on DVE.
  - The attention-value matmul uses pt blocks as the stationary operand so the
    output lands as ctx[q partitions, d free] with a ones-column denominator:
    full 128-partition output halves the PE row count vs the [d, q] layout.
  - Wv is folded into Wo on the host (Wo' = Wo @ blockdiag(Wv)) so no V
    projection runs on device; normalization is a per-partition reciprocal
    plus a free-dim broadcast multiply straight into concat staging.
  - concat [q, e] is flipped to [e, q] via PE transposes against a host
    identity, then the output projection streams q rows per 128-q chunk.
"""
import sys
for _p in ('/root/.axon_site/_ro/trn_rl_repo', '/opt/trn_rl_repo'):
    if _p not in sys.path:
        sys.path.append(_p)

import numpy as np
import ml_dtypes

import concourse.bass as bass
import concourse.tile as tile
from concourse import bacc, mybir
from concourse import bass_utils

F32 = mybir.dt.float32
BF16 = mybir.dt.bfloat16
I16 = mybir.dt.int16
AF = mybir.ActivationFunctionType
ALU = mybir.AluOpType

N_CORES = 8
B, S, E, H, DH = 2, 4096, 512, 8, 64
QLEN = S // 4          # 1024 q rows per core
KC = S // 128          # 32 k chunks
QW = QLEN // 512       # 2 q windows of 512

# Schraudolph exp-as-bf16-bits: int16(x*EXPA + EXPB) bitcast bf16 ~ exp(x/8)
LOG2E = 1.4426950408889634
EXPA = 128.0 * LOG2E / 8.0
EXPB = 128.0 * (127.0 - 0.05735) + 0.5  # +0.5 compensates trunc-toward-zero

# exp engine split per 32-chunk window: 'a'=ACT exact, 'p'=Pool, 'v'=DVE.
# Pool exps sit at kc>=8 so Pool is idle at window boundaries and can run
# the normalize ops of the previous window immediately.
EXP_ENG = {}
_pool_kcs = {8, 10, 12, 14, 16, 18, 20, 22, 24, 26, 28, 30}
_dve_kcs = {31}
for _kc in range(KC):
    EXP_ENG[_kc] = 'p' if _kc in _pool_kcs else ('v' if _kc in _dve_kcs else 'a')

_CACHE = {}


def _build_module():
    nc = bacc.Bacc("TRN2", target_bir_lowering=False, debug=False,
                   enable_asserts=True, num_devices=N_CORES)

    xqT = nc.dram_tensor("xqT", [E, QLEN], BF16, kind="ExternalInput").ap()
    xkT = nc.dram_tensor("xkT", [E, S], BF16, kind="ExternalInput").ap()
    valp = nc.dram_tensor("valp", [S, H * 65], BF16, kind="ExternalInput").ap()
    maskT = nc.dram_tensor("maskT", [S, QLEN], BF16, kind="ExternalInput").ap()
    ident = nc.dram_tensor("ident", [128, 128], BF16, kind="ExternalInput").ap()
    wqT = nc.dram_tensor("wqT", [DH, DH], BF16, kind="ExternalInput").ap()
    wkT = nc.dram_tensor("wkT", [DH, DH], BF16, kind="ExternalInput").ap()
    woT = nc.dram_tensor("woT", [E, E], BF16, kind="ExternalInput").ap()
    bo_b = nc.dram_tensor("bo_b", [128, E], F32, kind="ExternalInput").ap()
    out = nc.dram_tensor("out", [QLEN, E], F32, kind="ExternalOutput").ap()

    with tile.TileContext(nc) as tc:
        _emit(tc, nc, xqT, xkT, valp, maskT, ident, wqT, wkT, woT, bo_b, out)

    nc.compile()
    return nc


def _emit(tc, nc, xqT, xkT, valp, maskT, ident, wqT, wkT, woT, bo_b, out):
    from contextlib import ExitStack
    ctx = ExitStack()
    const = ctx.enter_context(tc.tile_pool(name="const", bufs=1))
    kpool = ctx.enter_context(tc.tile_pool(name="kproj", bufs=1))
    qpool = ctx.enter_context(tc.tile_pool(name="qproj", bufs=2))
    xkst = ctx.enter_context(tc.tile_pool(name="xkst", bufs=2))
    ppool = ctx.enter_context(tc.tile_pool(name="p", bufs=8))
    rcpool = ctx.enter_context(tc.tile_pool(name="rc", bufs=2))
    ospool = ctx.enter_context(tc.tile_pool(name="osb", bufs=2))
    psp = ctx.enter_context(tc.tile_pool(name="psp", bufs=3, space="PSUM"))
    uacc = ctx.enter_context(tc.tile_pool(name="uacc", bufs=1, space="PSUM"))

    def pstile(nm):
        """Scratch PSUM [128, 512] carved from the shared ps rotation."""
        t = psp.tile([128, 1024], F32, tag="ps", name=nm)
        return t[:, 0:512]

    # ---- resident mask tiles, 4 k-chunks per tile (loaded once, batched
    # DMAs: HWDGE descriptor generation is ~630ns per dma_start, so window 0
    # can't afford one DMA per 128-row chunk) ----
    mask_res = [const.tile([128, 4 * QLEN], BF16, tag=f"mk{c}", name=f"mk{c}")
                for c in range(KC // 4)]

    def mask_ap(kc, qw):
        t = mask_res[kc // 4]
        return t[:, (kc % 4) * QLEN + qw * 512:(kc % 4) * QLEN + qw * 512 + 512]

    def load_masks():
        for c in range(KC // 4):
            dst = mask_res[c].rearrange("p (c q) -> p c q", c=4)
            src = bass.AP(tensor=maskT.tensor, offset=c * 512 * QLEN,
                          ap=[[QLEN, 128], [128 * QLEN, 4], [1, QLEN]])
            nc.sync.dma_start(dst, src)

    # ---- constants (wq/wk immediately; heavy/late consts after proj0 loads)
    wq_sb = const.tile([DH, DH], BF16, tag="wq")
    nc.gpsimd.dma_start(wq_sb, wqT)
    wk_sb = const.tile([DH, DH], BF16, tag="wk")
    nc.gpsimd.dma_start(wk_sb, wkT)
    ident_sb = const.tile([128, 128], BF16, tag="ident")
    nc.gpsimd.dma_start(ident_sb, ident)
    wo_sb = []
    for pc in range(4):
        wo_sb.append(const.tile([128, E], BF16, tag=f"wo{pc}", name=f"wo{pc}"))
    bo_sb = const.tile([128, E], F32, tag="bo")

    def load_late_consts():
        for pc in range(4):
            nc.gpsimd.dma_start(wo_sb[pc], woT[pc * 128:(pc + 1) * 128, :])
        nc.gpsimd.dma_start(bo_sb, bo_b)

    # valp, 4 k-chunks per tile, batched DMAs on the ACT HWDGE queue
    VW = H * 65
    valp_t = []
    for c in range(KC // 4):
        t = const.tile([128, 4 * VW], BF16, tag=f"vp{c}", name=f"vp{c}")
        dst = t.rearrange("p (c v) -> p c v", c=4)
        src = bass.AP(tensor=valp.tensor, offset=c * 512 * VW,
                      ap=[[VW, 128], [128 * VW, 4], [1, VW]])
        nc.scalar.dma_start(dst, src)
        valp_t.append(t)

    def valp_ap(kc, h):
        t = valp_t[kc // 4]
        return t[:, (kc % 4) * VW + h * 65:(kc % 4) * VW + h * 65 + 65]

    # concat staging [q-chunk 128, E] bf16, one per 128-q chunk of this core
    concat_q = [const.tile([128, E], BF16, tag=f"cq{c}", name=f"cq{c}")
                for c in range(8)]
    # transposed concat [e, q] for the out-projection lhsT: 4 pc-blocks x QLEN
    ct_sb = const.tile([128, 4 * QLEN], BF16, tag="ct")

    kproj_sb = [None] * 4
    qproj_sb = [None] * 4
    xs = {}

    def proj_load(pair):
        # pair 0 is startup-critical (sync + scalar HWDGE); pair 1 rides the
        # gpsimd SWDGE queue before Pool's exp work begins; pairs 2-3 use the
        # scalar queue, which is empty by then.
        qk = nc.sync if pair == 0 else (nc.gpsimd if pair == 1 else nc.scalar)
        qq = nc.scalar if pair == 0 else (nc.gpsimd if pair == 1 else nc.scalar)
        xk0 = xkst.tile([DH, S], BF16, tag="xk", name=f"xk0_{pair}")
        qk.dma_start(xk0, xkT[(2 * pair) * DH:(2 * pair + 1) * DH, :])
        xq0 = xkst.tile([DH, QLEN], BF16, tag="xq", name=f"xq0_{pair}")
        qq.dma_start(xq0, xqT[(2 * pair) * DH:(2 * pair + 1) * DH, :])
        xk1 = xkst.tile([DH, S], BF16, tag="xk", name=f"xk1_{pair}")
        qk.dma_start(xk1, xkT[(2 * pair + 1) * DH:(2 * pair + 2) * DH, :])
        xq1 = xkst.tile([DH, QLEN], BF16, tag="xq", name=f"xq1_{pair}")
        qq.dma_start(xq1, xqT[(2 * pair + 1) * DH:(2 * pair + 2) * DH, :])
        xs[pair] = (xk0, xk1, xq0, xq1)
        kproj_sb[pair] = kpool.tile([128, S], BF16, tag=f"kp{pair}",
                                    name=f"kp{pair}")
        qproj_sb[pair] = qpool.tile([128, QLEN], BF16, tag="qp",
                                    name=f"qp{pair}")

    def proj_chunks(pair, fast_start=False):
        """Closures: 8 kproj chunks + 2 qproj chunks. Copies run on ACT."""
        xk0, xk1, xq0, xq1 = xs[pair]
        kp_sb = kproj_sb[pair]
        qp_sb = qproj_sb[pair]

        def half(dst, srcs, w, h2, nm):
            def go():
                t = pstile(nm)
                lo, hi = h2 * 64, (h2 + 1) * 64
                nc.tensor.matmul(t[lo:hi, :], lhsT=w, rhs=srcs[:, 0:512],
                                 start=True, stop=True)
                nc.scalar.copy(dst[lo:hi, 0:512], t[lo:hi, :])
            return go

        def kchunk(kc):
            def go():
                kp = pstile(f"kpp{pair}_{kc}")
                nc.tensor.matmul(kp[0:64, :], lhsT=wk_sb,
                                 rhs=xk0[:, kc * 512:(kc + 1) * 512],
                                 start=True, stop=True)
                nc.tensor.matmul(kp[64:128, :], lhsT=wk_sb,
                                 rhs=xk1[:, kc * 512:(kc + 1) * 512],
                                 start=True, stop=True)
                nc.scalar.copy(kp_sb[:, kc * 512:(kc + 1) * 512], kp)
            return go

        def qchunk(qc):
            def go():
                qp = pstile(f"qpp{pair}_{qc}")
                nc.tensor.matmul(qp[0:64, :], lhsT=wq_sb,
                                 rhs=xq0[:, qc * 512:(qc + 1) * 512],
                                 start=True, stop=True)
                nc.tensor.matmul(qp[64:128, :], lhsT=wq_sb,
                                 rhs=xq1[:, qc * 512:(qc + 1) * 512],
                                 start=True, stop=True)
                nc.scalar.copy(qp_sb[:, qc * 512:(qc + 1) * 512], qp)
            return go

        if fast_start:
            return ([half(kp_sb, xk0, wk_sb, 0, "fk0"),
                     half(qp_sb, xq0, wq_sb, 0, "fq0"),
                     half(kp_sb, xk1, wk_sb, 1, "fk1"),
                     half(qp_sb, xq1, wq_sb, 1, "fq1")]
                    + [kchunk(kc) for kc in range(1, 8)]
                    + [qchunk(qc) for qc in range(1, QW)])
        return ([kchunk(0), qchunk(0)] + [kchunk(kc) for kc in range(1, 8)]
                + [qchunk(qc) for qc in range(1, QW)])

    def attn(pair, qw, trickle=()):
        """One (head-pair, q-window): scores -> exp -> mask -> PV accumulate.

        Superseded by the global pipeline below; kept out of use."""
        raise NotImplementedError

    def finish_chunk(c):
        """Transpose concat chunk c into ct and run its out-projection."""
        def tr():
            ut = pstile(f"tr{c}")
            ut_bf = ut.bitcast(BF16)
            for pc in range(4):
                nc.tensor.transpose(ut_bf[:, pc * 128:(pc + 1) * 128],
                                    concat_q[c][:, pc * 128:(pc + 1) * 128],
                                    ident_sb)
            ctv = bass.AP(tensor=ct_sb.tensor, offset=ct_sb.offset + c * 128,
                          ap=[ct_sb.ap[0], [QLEN, 4], [1, 128]])
            nc.vector.tensor_copy(ctv, ut_bf[:, 0:512])

        def op():
            o = pstile(f"op{c}")
            for pc in range(4):
                nc.tensor.matmul(o,
                                 lhsT=ct_sb[:, pc * QLEN + c * 128:
                                            pc * QLEN + (c + 1) * 128],
                                 rhs=wo_sb[pc],
                                 start=(pc == 0), stop=(pc == 3))
            osb = ospool.tile([128, E], F32, tag="osb", name=f"osb{c}")
            nc.vector.scalar_tensor_tensor(osb, o, 1.0, bo_sb,
                                           ALU.mult, ALU.add)
            nc.sync.dma_start(out[c * 128:(c + 1) * 128, :], osb)
        return [tr, op]

    # ---- global attention pipeline -------------------------------------
    # One continuous stream of 256 chunks (8 windows x 32 k-chunks); scores/
    # exp/mask run SKEW chunks ahead of the PV accumulation so neither PE nor
    # the elementwise engines ever wait on the in-flight chunk, including
    # across window boundaries.
    from collections import deque
    WINDOWS = [(p, w) for p in range(4) for w in range(QW)]
    SKEW = 6
    NG = len(WINDOWS) * KC
    Umap = {}
    pts = {}

    def emit_scores(g):
        w, kc = divmod(g, KC)
        pair, qw = WINDOWS[w]
        kp_sb = kproj_sb[pair]
        qp_sb = qproj_sb[pair]
        ps = psp.tile([128, 1024], F32, tag="ps", name=f"ps{w}_{kc}")
        nc.tensor.matmul(
            ps[:, 0:512],
            lhsT=kp_sb[0:64, kc * 128:(kc + 1) * 128],
            rhs=qp_sb[0:64, qw * 512:(qw + 1) * 512],
            start=True, stop=True)
        nc.tensor.matmul(
            ps[:, 512:1024],
            lhsT=kp_sb[64:128, kc * 128:(kc + 1) * 128],
            rhs=qp_sb[64:128, qw * 512:(qw + 1) * 512],
            start=True, stop=True)
        pt = ppool.tile([128, 1024], BF16, tag="pt", name=f"pt{w}_{kc}")
        pts[g] = pt
        eng = EXP_ENG[kc]
        if eng == 'a':
            nc.scalar.activation(pt, ps, AF.Exp, bias=0.0, scale=0.125)
        else:
            e = nc.vector if eng == 'v' else nc.gpsimd
            e.tensor_scalar(pt.bitcast(I16), ps, EXPA, EXPB,
                            ALU.mult, ALU.add)
        # mask multiply (DVE, bf16 2x), broadcast across the head pair
        ms = mask_ap(kc, qw)
        mb = bass.AP(tensor=ms.tensor, offset=ms.offset,
                     ap=[ms.ap[0], [0, 2], [1, 512]])
        pv = pt.rearrange("p (h q) -> p h q", h=2)
        nc.vector.tensor_mul(pv, pv, mb)

    def emit_norms(w):
        """Normalize window w's context into concat staging. Runs on Pool
        (idle at window boundaries) as a single divide per (head, qsub):
        out = ctx_cols / denominator_col, broadcast from PSUM."""
        pair, qw = WINDOWS[w]
        U = Umap[w]
        for h2 in range(2):
            h = 2 * pair + h2
            for qs in range(4):
                den = bass.AP(tensor=U[h2].tensor,
                              offset=U[h2].offset + qs * 65 + 64,
                              ap=[U[h2].ap[0], [0, 64]])
                nc.gpsimd.scalar_tensor_tensor(
                    concat_q[qw * 4 + qs][:, h * 64:(h + 1) * 64],
                    U[h2][:, qs * 65:qs * 65 + 64], 1.0, den,
                    ALU.mult, ALU.divide)

    def emit_pv(g):
        w, kc = divmod(g, KC)
        pair, qw = WINDOWS[w]
        if kc == 0:
            Umap[w] = [uacc.tile([128, 260], F32, tag=f"u{h2}",
                                 name=f"U{w}_{h2}")
                       for h2 in range(2)]
        U = Umap[w]
        pt = pts.pop(g)
        # One start=True per U bank marks the whole bank pending-zero; each
        # group's first write then zero-fills its own bytes.
        for h2 in range(2):
            h = 2 * pair + h2
            for qs in range(4):
                nc.tensor.matmul(
                    U[h2][:, qs * 65:qs * 65 + 65],
                    lhsT=pt[:, h2 * 512 + qs * 128:h2 * 512 + (qs + 1) * 128],
                    rhs=valp_ap(kc, h),
                    start=(kc == 0 and qs == 0), stop=(kc == KC - 1),
                    skip_group_check=True)
        if kc == KC - 1:
            emit_norms(w)

    # startup: pair-0/1 loads, first projection halves inline, rest in aux
    proj_load(0)
    load_masks()
    proj_load(1)
    load_late_consts()
    fs = proj_chunks(0, fast_start=True)
    fs[0]()  # fk0 (k cols 0:512, both heads)
    fs[1]()  # fq0 (q window 0)
    aux = deque(fs[2:])
    aux.extend(proj_chunks(1))
    for g in range(NG + SKEW):
        if g == 2 * KC:
            proj_load(2)
            aux.extend(proj_chunks(2))
        elif g == 4 * KC:
            proj_load(3)
            aux.extend(proj_chunks(3))
        if g < NG:
            emit_scores(g)
        if g >= SKEW:
            emit_pv(g - SKEW)
            wv, kcv = divmod(g - SKEW, KC)
            if kcv == KC - 1 and WINDOWS[wv][0] == 3:
                qwv = WINDOWS[wv][1]
                for c in range(qwv * 4, qwv * 4 + 4):
                    aux.extend(finish_chunk(c))
        if aux:
            aux.popleft()()
    while aux:
        aux.popleft()()

    ctx.close()


def _prep_inputs(key, query, value, mask, Wq, Wk, Wv, Wo, bo):
    bf16 = ml_dtypes.bfloat16
    key = np.asarray(key, np.float32)
    query = np.asarray(query, np.float32)
    value = np.asarray(value, np.float32)
    mask = np.asarray(mask)
    Wv = np.asarray(Wv, np.float32)
    Wo = np.asarray(Wo, np.float32)
    # fold the V projection into the output projection:
    # concat_h(ctxraw_h @ Wv^T) @ Wo^T == concat_raw @ (Wo @ blockdiag(Wv))^T
    Wof = np.empty_like(Wo)
    for h in range(H):
        Wof[:, h * DH:(h + 1) * DH] = Wo[:, h * DH:(h + 1) * DH] @ Wv
    common = {
        "wqT": np.ascontiguousarray(np.asarray(Wq, np.float32).T).astype(bf16),
        "wkT": np.ascontiguousarray(np.asarray(Wk, np.float32).T).astype(bf16),
        "woT": np.ascontiguousarray(Wof.T).astype(bf16),
        "bo_b": np.ascontiguousarray(
            np.broadcast_to(np.asarray(bo, np.float32), (128, E))),
        "ident": np.eye(128, dtype=np.float32).astype(bf16),
    }
    maskT = np.ascontiguousarray(
        (mask[0, 0] != 0).astype(np.float32).T.astype(bf16))  # [k, q]
    per_b = {}
    for b in range(B):
        vp = np.ones((S, H, 65), np.float32)
        vp[:, :, :64] = value[b].reshape(S, H, DH)
        per_b[b] = {
            "xkT": np.ascontiguousarray(key[b].T).astype(bf16),
            "valp": np.ascontiguousarray(vp.reshape(S, H * 65).astype(bf16)),
            "qT": query[b].T,
        }
    in_maps = []
    for c in range(N_CORES):
        b, qs = c // 4, (c % 4) * QLEN
        in_maps.append({
            "xqT": np.ascontiguousarray(
                per_b[b]["qT"][:, qs:qs + QLEN]).astype(bf16),
            "xkT": per_b[b]["xkT"],
            "valp": per_b[b]["valp"],
            "maskT": np.ascontiguousarray(maskT[:, qs:qs + QLEN]),
            **common,
        })
    return in_maps


def get_module():
    if "nc" not in _CACHE:
        _CACHE["nc"] = _build_module()
    return _CACHE["nc"]


def kernel(key, query, value, mask, Wq, Wk, Wv, Wo, bo, **_):
    nc = get_module()
    in_maps = _prep_inputs(key, query, value, mask, Wq, Wk, Wv, Wo, bo)
    res = bass_utils.run_bass_kernel_spmd(
        nc, in_maps, core_ids=list(range(N_CORES)))
    full = np.empty((B, S, E), np.float32)
    for c in range(N_CORES):
        b, qs = c // 4, (c % 4) * QLEN
        full[b, qs:qs + QLEN, :] = res.results[c]["out"]
    return full


# revision 24
# speedup vs baseline: 1.1985x; 1.0162x over previous
"""Trainium2 Bass kernel for 8-head MultiHeadAttention (B=2, S=4096, E=512).

Sharding: 8 cores = 2 batches x 4 query-row chunks of 1024. Each core computes
all 8 heads for its (batch, q-range). Structure:
  - QK^T scores built transposed ([k partitions, q free]) as in the baseline.
  - softmax exp split across three engines: ACT (exact table exp) plus DVE and
    GPSIMD using a single-instruction Schraudolph bit-trick (int16 write
    bitcast to bf16), all masked multiplicatively afterward on DVE.
  - The attention-value matmul uses pt blocks as the stationary operand so the
    output lands as ctx[q partitions, d free] with a ones-column denominator:
    full 128-partition output halves the PE row count vs the [d, q] layout.
  - Wv is folded into Wo on the host (Wo' = Wo @ blockdiag(Wv)) so no V
    projection runs on device; normalization is a per-partition reciprocal
    plus a free-dim broadcast multiply straight into concat staging.
  - concat [q, e] is flipped to [e, q] via PE transposes against a host
    identity, then the output projection streams q rows per 128-q chunk.
"""
import sys
for _p in ('/root/.axon_site/_ro/trn_rl_repo', '/opt/trn_rl_repo'):
    if _p not in sys.path:
        sys.path.append(_p)

import numpy as np
import ml_dtypes

import concourse.bass as bass
import concourse.tile as tile
from concourse import bacc, mybir
from concourse import bass_utils

F32 = mybir.dt.float32
BF16 = mybir.dt.bfloat16
I16 = mybir.dt.int16
AF = mybir.ActivationFunctionType
ALU = mybir.AluOpType

N_CORES = 8
B, S, E, H, DH = 2, 4096, 512, 8, 64
QLEN = S // 4          # 1024 q rows per core
KC = S // 128          # 32 k chunks
QW = QLEN // 512       # 2 q windows of 512

# Schraudolph exp-as-bf16-bits: int16(x*EXPA + EXPB) bitcast bf16 ~ exp(x/8)
LOG2E = 1.4426950408889634
EXPA = 128.0 * LOG2E / 8.0
EXPB = 128.0 * (127.0 - 0.05735) + 0.5  # +0.5 compensates trunc-toward-zero

# exp engine split per 32-chunk window: 'a'=ACT exact, 'p'=Pool, 'v'=DVE.
# Pool exps sit at kc>=8 so Pool is idle at window boundaries and can run
# the normalize ops of the previous window immediately.
EXP_ENG = {}
_pool_kcs = {8, 10, 12, 14, 16, 18, 20, 22, 24, 26, 28, 30}
_dve_kcs = {31}
for _kc in range(KC):
    EXP_ENG[_kc] = 'p' if _kc in _pool_kcs else ('v' if _kc in _dve_kcs else 'a')

_CACHE = {}


def _build_module():
    nc = bacc.Bacc("TRN2", target_bir_lowering=False, debug=False,
                   enable_asserts=True, num_devices=N_CORES)

    xqT = nc.dram_tensor("xqT", [E, QLEN], BF16, kind="ExternalInput").ap()
    xkT = nc.dram_tensor("xkT", [E, S], BF16, kind="ExternalInput").ap()
    valp = nc.dram_tensor("valp", [S, H * 65], BF16, kind="ExternalInput").ap()
    maskT = nc.dram_tensor("maskT", [S, QLEN], BF16, kind="ExternalInput").ap()
    ident = nc.dram_tensor("ident", [128, 128], BF16, kind="ExternalInput").ap()
    wqT = nc.dram_tensor("wqT", [DH, DH], BF16, kind="ExternalInput").ap()
    wkT = nc.dram_tensor("wkT", [DH, DH], BF16, kind="ExternalInput").ap()
    woT = nc.dram_tensor("woT", [E, E], BF16, kind="ExternalInput").ap()
    bo_b = nc.dram_tensor("bo_b", [128, E], F32, kind="ExternalInput").ap()
    out = nc.dram_tensor("out", [QLEN, E], F32, kind="ExternalOutput").ap()

    with tile.TileContext(nc) as tc:
        _emit(tc, nc, xqT, xkT, valp, maskT, ident, wqT, wkT, woT, bo_b, out)

    nc.compile()
    return nc


def _emit(tc, nc, xqT, xkT, valp, maskT, ident, wqT, wkT, woT, bo_b, out):
    from contextlib import ExitStack
    ctx = ExitStack()
    const = ctx.enter_context(tc.tile_pool(name="const", bufs=1))
    kpool = ctx.enter_context(tc.tile_pool(name="kproj", bufs=1))
    qpool = ctx.enter_context(tc.tile_pool(name="qproj", bufs=2))
    xkst = ctx.enter_context(tc.tile_pool(name="xkst", bufs=2))
    ppool = ctx.enter_context(tc.tile_pool(name="p", bufs=10))
    rcpool = ctx.enter_context(tc.tile_pool(name="rc", bufs=2))
    ospool = ctx.enter_context(tc.tile_pool(name="osb", bufs=2))
    psp = ctx.enter_context(tc.tile_pool(name="psp", bufs=3, space="PSUM"))
    uacc = ctx.enter_context(tc.tile_pool(name="uacc", bufs=1, space="PSUM"))

    def pstile(nm):
        """Scratch PSUM [128, 512] carved from the shared ps rotation."""
        t = psp.tile([128, 1024], F32, tag="ps", name=nm)
        return t[:, 0:512]

    # ---- resident mask tiles, 4 k-chunks per tile (loaded once, batched
    # DMAs: HWDGE descriptor generation is ~630ns per dma_start, so window 0
    # can't afford one DMA per 128-row chunk) ----
    mask_res = [const.tile([128, 4 * QLEN], BF16, tag=f"mk{c}", name=f"mk{c}")
                for c in range(KC // 4)]

    def mask_ap(kc, qw):
        t = mask_res[kc // 4]
        return t[:, (kc % 4) * QLEN + qw * 512:(kc % 4) * QLEN + qw * 512 + 512]

    def load_masks():
        for c in range(KC // 4):
            dst = mask_res[c].rearrange("p (c q) -> p c q", c=4)
            src = bass.AP(tensor=maskT.tensor, offset=c * 512 * QLEN,
                          ap=[[QLEN, 128], [128 * QLEN, 4], [1, QLEN]])
            nc.sync.dma_start(dst, src)

    # ---- constants (wq/wk immediately; heavy/late consts after proj0 loads)
    wq_sb = const.tile([DH, DH], BF16, tag="wq")
    nc.gpsimd.dma_start(wq_sb, wqT)
    wk_sb = const.tile([DH, DH], BF16, tag="wk")
    nc.gpsimd.dma_start(wk_sb, wkT)
    ident_sb = const.tile([128, 128], BF16, tag="ident")
    nc.gpsimd.dma_start(ident_sb, ident)
    wo_sb = []
    for pc in range(4):
        wo_sb.append(const.tile([128, E], BF16, tag=f"wo{pc}", name=f"wo{pc}"))
    bo_sb = const.tile([128, E], F32, tag="bo")

    def load_late_consts():
        for pc in range(4):
            nc.gpsimd.dma_start(wo_sb[pc], woT[pc * 128:(pc + 1) * 128, :])
        nc.gpsimd.dma_start(bo_sb, bo_b)

    # valp, 4 k-chunks per tile, batched DMAs on the ACT HWDGE queue
    VW = H * 65
    valp_t = [const.tile([128, 4 * VW], BF16, tag=f"vp{c}", name=f"vp{c}")
              for c in range(KC // 4)]

    def load_valp():
        for c in range(KC // 4):
            dst = valp_t[c].rearrange("p (c v) -> p c v", c=4)
            src = bass.AP(tensor=valp.tensor, offset=c * 512 * VW,
                          ap=[[VW, 128], [128 * VW, 4], [1, VW]])
            nc.scalar.dma_start(dst, src)

    def valp_ap(kc, h):
        t = valp_t[kc // 4]
        return t[:, (kc % 4) * VW + h * 65:(kc % 4) * VW + h * 65 + 65]

    # concat staging [q-chunk 128, E] bf16, one per 128-q chunk of this core
    concat_q = [const.tile([128, E], BF16, tag=f"cq{c}", name=f"cq{c}")
                for c in range(8)]
    # transposed concat [e, q] for the out-projection lhsT: 4 pc-blocks x QLEN
    ct_sb = const.tile([128, 4 * QLEN], BF16, tag="ct")

    kproj_sb = [None] * 4
    qproj_sb = [None] * 4
    xs = {}

    def proj_load(pair):
        # pair 0 is startup-critical (sync + scalar HWDGE); pair 1 rides the
        # gpsimd SWDGE queue before Pool's exp work begins; pairs 2-3 use the
        # scalar queue, which is empty by then.
        qk = nc.sync if pair == 0 else (nc.gpsimd if pair == 1 else nc.scalar)
        qq = nc.scalar if pair == 0 else (nc.gpsimd if pair == 1 else nc.scalar)
        xk0 = xkst.tile([DH, S], BF16, tag="xk", name=f"xk0_{pair}")
        qk.dma_start(xk0, xkT[(2 * pair) * DH:(2 * pair + 1) * DH, :])
        xq0 = xkst.tile([DH, QLEN], BF16, tag="xq", name=f"xq0_{pair}")
        qq.dma_start(xq0, xqT[(2 * pair) * DH:(2 * pair + 1) * DH, :])
        xk1 = xkst.tile([DH, S], BF16, tag="xk", name=f"xk1_{pair}")
        qk.dma_start(xk1, xkT[(2 * pair + 1) * DH:(2 * pair + 2) * DH, :])
        xq1 = xkst.tile([DH, QLEN], BF16, tag="xq", name=f"xq1_{pair}")
        qq.dma_start(xq1, xqT[(2 * pair + 1) * DH:(2 * pair + 2) * DH, :])
        xs[pair] = (xk0, xk1, xq0, xq1)
        kproj_sb[pair] = kpool.tile([128, S], BF16, tag=f"kp{pair}",
                                    name=f"kp{pair}")
        qproj_sb[pair] = qpool.tile([128, QLEN], BF16, tag="qp",
                                    name=f"qp{pair}")

    def proj_chunks(pair, fast_start=False):
        """Closures: 8 kproj chunks + 2 qproj chunks. Copies run on ACT."""
        xk0, xk1, xq0, xq1 = xs[pair]
        kp_sb = kproj_sb[pair]
        qp_sb = qproj_sb[pair]

        def half(dst, srcs, w, h2, nm):
            def go():
                t = pstile(nm)
                lo, hi = h2 * 64, (h2 + 1) * 64
                nc.tensor.matmul(t[lo:hi, :], lhsT=w, rhs=srcs[:, 0:512],
                                 start=True, stop=True)
                nc.scalar.copy(dst[lo:hi, 0:512], t[lo:hi, :])
            return go

        def kchunk(kc):
            def go():
                kp = pstile(f"kpp{pair}_{kc}")
                nc.tensor.matmul(kp[0:64, :], lhsT=wk_sb,
                                 rhs=xk0[:, kc * 512:(kc + 1) * 512],
                                 start=True, stop=True)
                nc.tensor.matmul(kp[64:128, :], lhsT=wk_sb,
                                 rhs=xk1[:, kc * 512:(kc + 1) * 512],
                                 start=True, stop=True)
                nc.scalar.copy(kp_sb[:, kc * 512:(kc + 1) * 512], kp)
            return go

        def qchunk(qc):
            def go():
                qp = pstile(f"qpp{pair}_{qc}")
                nc.tensor.matmul(qp[0:64, :], lhsT=wq_sb,
                                 rhs=xq0[:, qc * 512:(qc + 1) * 512],
                                 start=True, stop=True)
                nc.tensor.matmul(qp[64:128, :], lhsT=wq_sb,
                                 rhs=xq1[:, qc * 512:(qc + 1) * 512],
                                 start=True, stop=True)
                nc.scalar.copy(qp_sb[:, qc * 512:(qc + 1) * 512], qp)
            return go

        if fast_start:
            return ([half(kp_sb, xk0, wk_sb, 0, "fk0"),
                     half(qp_sb, xq0, wq_sb, 0, "fq0"),
                     half(kp_sb, xk1, wk_sb, 1, "fk1"),
                     half(qp_sb, xq1, wq_sb, 1, "fq1")]
                    + [kchunk(kc) for kc in range(1, 8)]
                    + [qchunk(qc) for qc in range(1, QW)])
        return ([kchunk(0), qchunk(0)] + [kchunk(kc) for kc in range(1, 8)]
                + [qchunk(qc) for qc in range(1, QW)])

    def attn(pair, qw, trickle=()):
        """One (head-pair, q-window): scores -> exp -> mask -> PV accumulate.

        Superseded by the global pipeline below; kept out of use."""
        raise NotImplementedError

    def finish_chunk(c):
        """Transpose concat chunk c into ct and run its out-projection."""
        def tr():
            ut = pstile(f"tr{c}")
            ut_bf = ut.bitcast(BF16)
            for pc in range(4):
                nc.tensor.transpose(ut_bf[:, pc * 128:(pc + 1) * 128],
                                    concat_q[c][:, pc * 128:(pc + 1) * 128],
                                    ident_sb)
            ctv = bass.AP(tensor=ct_sb.tensor, offset=ct_sb.offset + c * 128,
                          ap=[ct_sb.ap[0], [QLEN, 4], [1, 128]])
            nc.vector.tensor_copy(ctv, ut_bf[:, 0:512])

        def op():
            o = pstile(f"op{c}")
            for pc in range(4):
                nc.tensor.matmul(o,
                                 lhsT=ct_sb[:, pc * QLEN + c * 128:
                                            pc * QLEN + (c + 1) * 128],
                                 rhs=wo_sb[pc],
                                 start=(pc == 0), stop=(pc == 3))
            osb = ospool.tile([128, E], F32, tag="osb", name=f"osb{c}")
            nc.vector.scalar_tensor_tensor(osb, o, 1.0, bo_sb,
                                           ALU.mult, ALU.add)
            nc.sync.dma_start(out[c * 128:(c + 1) * 128, :], osb)
        return [tr, op]

    # ---- global attention pipeline -------------------------------------
    # One continuous stream of 256 chunks (8 windows x 32 k-chunks); scores/
    # exp/mask run SKEW chunks ahead of the PV accumulation so neither PE nor
    # the elementwise engines ever wait on the in-flight chunk, including
    # across window boundaries.
    from collections import deque
    WINDOWS = [(p, w) for p in range(4) for w in range(QW)]
    SKEW = 8
    NG = len(WINDOWS) * KC
    Umap = {}
    pts = {}

    def emit_scores(g):
        w, kc = divmod(g, KC)
        pair, qw = WINDOWS[w]
        kp_sb = kproj_sb[pair]
        qp_sb = qproj_sb[pair]
        ps = psp.tile([128, 1024], F32, tag="ps", name=f"ps{w}_{kc}")
        nc.tensor.matmul(
            ps[:, 0:512],
            lhsT=kp_sb[0:64, kc * 128:(kc + 1) * 128],
            rhs=qp_sb[0:64, qw * 512:(qw + 1) * 512],
            start=True, stop=True)
        nc.tensor.matmul(
            ps[:, 512:1024],
            lhsT=kp_sb[64:128, kc * 128:(kc + 1) * 128],
            rhs=qp_sb[64:128, qw * 512:(qw + 1) * 512],
            start=True, stop=True)
        pt = ppool.tile([128, 1024], BF16, tag="pt", name=f"pt{w}_{kc}")
        pts[g] = pt
        eng = EXP_ENG[kc]
        if eng == 'a':
            nc.scalar.activation(pt, ps, AF.Exp, bias=0.0, scale=0.125)
        else:
            e = nc.vector if eng == 'v' else nc.gpsimd
            e.tensor_scalar(pt.bitcast(I16), ps, EXPA, EXPB,
                            ALU.mult, ALU.add)
        # mask multiply (DVE, bf16 2x), broadcast across the head pair
        ms = mask_ap(kc, qw)
        mb = bass.AP(tensor=ms.tensor, offset=ms.offset,
                     ap=[ms.ap[0], [0, 2], [1, 512]])
        pv = pt.rearrange("p (h q) -> p h q", h=2)
        nc.vector.tensor_mul(pv, pv, mb)

    def emit_norms(w, inline_fins=False):
        """Normalize window w's context into concat staging. Runs on Pool
        (idle at window boundaries) as a single divide per (head, qsub):
        out = ctx_cols / denominator_col, broadcast from PSUM. For the last
        window the finish work is emitted eagerly per q-chunk."""
        pair, qw = WINDOWS[w]
        U = Umap[w]
        for qs in range(4):
            for h2 in range(2):
                h = 2 * pair + h2
                den = bass.AP(tensor=U[h2].tensor,
                              offset=U[h2].offset + qs * 65 + 64,
                              ap=[U[h2].ap[0], [0, 64]])
                nc.gpsimd.scalar_tensor_tensor(
                    concat_q[qw * 4 + qs][:, h * 64:(h + 1) * 64],
                    U[h2][:, qs * 65:qs * 65 + 64], 1.0, den,
                    ALU.mult, ALU.divide)
            if inline_fins:
                for work in finish_chunk(qw * 4 + qs):
                    work()

    def emit_pv(g):
        w, kc = divmod(g, KC)
        pair, qw = WINDOWS[w]
        if kc == 0:
            Umap[w] = [uacc.tile([128, 260], F32, tag=f"u{h2}",
                                 name=f"U{w}_{h2}")
                       for h2 in range(2)]
        U = Umap[w]
        pt = pts.pop(g)
        # One start=True per U bank marks the whole bank pending-zero; each
        # group's first write then zero-fills its own bytes.
        for h2 in range(2):
            h = 2 * pair + h2
            for qs in range(4):
                nc.tensor.matmul(
                    U[h2][:, qs * 65:qs * 65 + 65],
                    lhsT=pt[:, h2 * 512 + qs * 128:h2 * 512 + (qs + 1) * 128],
                    rhs=valp_ap(kc, h),
                    start=(kc == 0 and qs == 0), stop=(kc == KC - 1),
                    skip_group_check=True)
        if kc == KC - 1:
            emit_norms(w, inline_fins=(w == len(WINDOWS) - 1))

    # startup: pair-0 loads first (startup-critical), then masks/valp, then
    # pair-1; first projection halves inline, rest in aux
    proj_load(0)
    load_masks()
    load_valp()
    proj_load(1)
    load_late_consts()
    fs = proj_chunks(0, fast_start=True)
    fs[0]()  # fk0 (k cols 0:512, both heads)
    fs[1]()  # fq0 (q window 0)
    aux = deque(fs[2:])
    aux.extend(proj_chunks(1))
    for g in range(NG + SKEW):
        if g == 2 * KC:
            proj_load(2)
            aux.extend(proj_chunks(2))
        elif g == 4 * KC:
            proj_load(3)
            aux.extend(proj_chunks(3))
        if g < NG:
            emit_scores(g)
        if g >= SKEW:
            emit_pv(g - SKEW)
            wv, kcv = divmod(g - SKEW, KC)
            if kcv == KC - 1 and WINDOWS[wv] == (3, 0):
                for c in range(0, 4):
                    aux.extend(finish_chunk(c))
        if aux:
            aux.popleft()()
            if aux and g >= NG - 8:
                aux.popleft()()
    while aux:
        aux.popleft()()

    ctx.close()


def _prep_inputs(key, query, value, mask, Wq, Wk, Wv, Wo, bo):
    bf16 = ml_dtypes.bfloat16
    key = np.asarray(key, np.float32)
    query = np.asarray(query, np.float32)
    value = np.asarray(value, np.float32)
    mask = np.asarray(mask)
    Wv = np.asarray(Wv, np.float32)
    Wo = np.asarray(Wo, np.float32)
    # fold the V projection into the output projection:
    # concat_h(ctxraw_h @ Wv^T) @ Wo^T == concat_raw @ (Wo @ blockdiag(Wv))^T
    Wof = np.empty_like(Wo)
    for h in range(H):
        Wof[:, h * DH:(h + 1) * DH] = Wo[:, h * DH:(h + 1) * DH] @ Wv
    common = {
        "wqT": np.ascontiguousarray(np.asarray(Wq, np.float32).T).astype(bf16),
        "wkT": np.ascontiguousarray(np.asarray(Wk, np.float32).T).astype(bf16),
        "woT": np.ascontiguousarray(Wof.T).astype(bf16),
        "bo_b": np.ascontiguousarray(
            np.broadcast_to(np.asarray(bo, np.float32), (128, E))),
        "ident": np.eye(128, dtype=np.float32).astype(bf16),
    }
    maskT = np.ascontiguousarray(
        (mask[0, 0] != 0).astype(np.float32).T.astype(bf16))  # [k, q]
    per_b = {}
    for b in range(B):
        vp = np.ones((S, H, 65), np.float32)
        vp[:, :, :64] = value[b].reshape(S, H, DH)
        per_b[b] = {
            "xkT": np.ascontiguousarray(key[b].T).astype(bf16),
            "valp": np.ascontiguousarray(vp.reshape(S, H * 65).astype(bf16)),
            "qT": query[b].T,
        }
    in_maps = []
    for c in range(N_CORES):
        b, qs = c // 4, (c % 4) * QLEN
        in_maps.append({
            "xqT": np.ascontiguousarray(
                per_b[b]["qT"][:, qs:qs + QLEN]).astype(bf16),
            "xkT": per_b[b]["xkT"],
            "valp": per_b[b]["valp"],
            "maskT": np.ascontiguousarray(maskT[:, qs:qs + QLEN]),
            **common,
        })
    return in_maps


def get_module():
    if "nc" not in _CACHE:
        _CACHE["nc"] = _build_module()
    return _CACHE["nc"]


def kernel(key, query, value, mask, Wq, Wk, Wv, Wo, bo, **_):
    nc = get_module()
    in_maps = _prep_inputs(key, query, value, mask, Wq, Wk, Wv, Wo, bo)
    res = bass_utils.run_bass_kernel_spmd(
        nc, in_maps, core_ids=list(range(N_CORES)))
    full = np.empty((B, S, E), np.float32)
    for c in range(N_CORES):
        b, qs = c // 4, (c % 4) * QLEN
        full[b, qs:qs + QLEN, :] = res.results[c]["out"]
    return full
